# revision 2
# baseline (speedup 1.0000x reference)
"""Trainium2 Bass kernel for nn_CustomLSTMModel (B=64, S=512, D=256, H=1024).

Strategy: shard the HIDDEN dimension across the 8 NeuronCores (128 hidden
units per core). Each core computes its slice of the 4 gate projections
([64, 512] = 4 gates x 128 cols) for the full batch, which keeps every
engine's tiles at [64..128, *] shapes (full lane utilization), then updates
its slice of (C, H). The full hidden state H_t needed as the next step's
matmul stationary is reassembled every step with an 8-core AllGather of
each core's transposed H-shard ([128, 64] bf16, 16 KB).

Matmuls run in bf16 (fp32 PSUM accumulation); C stays fp32 locally.
End-to-end relative error vs the fp32 reference is ~3e-3.

The per-step x-projection is folded into the recurrent GEMM as two extra
K-chunks (x_t.T is pre-transposed on the host), and the final projection
pred = H @ Wd is computed in-loop as one tiny [1,64] matmul per step,
accumulated 8 steps per PSUM bank. Host code shards/permutes the inputs,
sums the per-core partial predictions, and concatenates Hf/Cf shards.
"""
import sys
sys.path.insert(0, '/opt/trn_rl_repo')
import numpy as np
import ml_dtypes
import concourse.bass as bass
import concourse.tile as tile
from concourse import mybir, bacc
from concourse.bass_utils import run_bass_kernel_spmd
from contextlib import ExitStack

B, S, D, H = 64, 512, 256, 1024
R = 8            # cores
NL = 512         # local gate cols (4 gates x 128)
HS = H // R      # hidden shard size
bf16 = mybir.dt.bfloat16
f32 = mybir.dt.float32


def _build(ncores=R, steps=S):
    nc = bacc.Bacc("TRN2", target_bir_lowering=False, debug=False, num_devices=ncores)
    wh = nc.dram_tensor("wh", [10, 128, NL], bf16, kind="ExternalInput").ap()
    bvec_d = nc.dram_tensor("bvec", [1, NL], bf16, kind="ExternalInput").ap()
    ones_d = nc.dram_tensor("ones", [1, B], bf16, kind="ExternalInput").ap()
    ident_d = nc.dram_tensor("ident", [B, B], f32, kind="ExternalInput").ap()
    wd_d = nc.dram_tensor("wd", [128, 1], bf16, kind="ExternalInput").ap()
    h0t_d = nc.dram_tensor("h0t", [128, 8 * B], bf16, kind="ExternalInput").ap()
    c0_d = nc.dram_tensor("c0", [B, HS], f32, kind="ExternalInput").ap()
    xt_d = nc.dram_tensor("xt", [S, 128, 2 * B], bf16, kind="ExternalInput").ap()
    hf_d = nc.dram_tensor("hf", [B, HS], f32, kind="ExternalOutput").ap()
    cf_d = nc.dram_tensor("cf", [B, HS], f32, kind="ExternalOutput").ap()
    predp_d = nc.dram_tensor("predp", [S // 8, 8 * B], f32, kind="ExternalOutput").ap()

    with tile.TileContext(nc) as tc:
        with ExitStack() as ctx:
            wpool = ctx.enter_context(tc.tile_pool(name="w", bufs=1))
            gpool = ctx.enter_context(tc.tile_pool(name="gath", bufs=3))
            xpool = ctx.enter_context(tc.tile_pool(name="xin", bufs=4))
            spool = ctx.enter_context(tc.tile_pool(name="state", bufs=1))
            tpool = ctx.enter_context(tc.tile_pool(name="tmp", bufs=3))
            htpool = ctx.enter_context(tc.tile_pool(name="ht", bufs=3))
            psG = ctx.enter_context(tc.tile_pool(name="psG", bufs=2, space="PSUM"))
            psT = ctx.enter_context(tc.tile_pool(name="psT", bufs=2, space="PSUM"))
            psP = ctx.enter_context(tc.tile_pool(name="psP", bufs=2, space="PSUM"))
            dpool = ctx.enter_context(tc.tile_pool(name="dram", bufs=1, space="DRAM"))
            dinpool = ctx.enter_context(tc.tile_pool(name="dramin", bufs=3, space="DRAM"))

            W = wpool.tile([128, 10 * NL], bf16)
            for j in range(10):
                nc.sync.dma_start(W[:, j * NL:(j + 1) * NL], wh[j])
            bvec = wpool.tile([1, NL], bf16)
            nc.sync.dma_start(bvec[:], bvec_d[:])
            ones_t = wpool.tile([1, B], bf16)
            nc.sync.dma_start(ones_t[:], ones_d[:])
            ident = wpool.tile([B, B], f32)
            nc.sync.dma_start(ident[:], ident_d[:])
            wd = wpool.tile([128, 1], bf16)
            nc.sync.dma_start(wd[:], wd_d[:])
            C = spool.tile([B, HS], f32)
            nc.sync.dma_start(C[:], c0_d[:])

            G = gpool.tile([128, 8 * B], bf16, tag="G", name="G_init")
            nc.sync.dma_start(G[:], h0t_d[:])

            ag_outs = [
                dpool.tile([8 * 128, B], bf16, addr_space="Shared", name=f"agout{t}")
                for t in range(steps - 1)
            ]

            Hn = None
            ppred = None
            for t in range(steps):
                HT = htpool.tile([128, B], bf16, tag="HT")
                ps = psG.tile([B, NL], f32, tag="psG")
                xtile = xpool.tile([128, 2 * B], bf16, tag="xt")
                nc.scalar.dma_start(xtile[:], xt_d[t])
                nc.tensor.matmul(ps[:], ones_t[:], bvec[:], start=True, stop=False)
                nc.tensor.matmul(ps[:], xtile[:, 0:B], W[:, 8 * NL:9 * NL],
                                 start=False, stop=False)
                nc.tensor.matmul(ps[:], xtile[:, B:2 * B], W[:, 9 * NL:10 * NL],
                                 start=False, stop=False)
                for j in range(8):
                    nc.tensor.matmul(ps[:], G[:, j * B:(j + 1) * B],
                                     W[:, j * NL:(j + 1) * NL],
                                     start=False, stop=(j == 7))
                sig = tpool.tile([B, 384], f32, tag="sig")
                ct = tpool.tile([B, 128], f32, tag="ct")
                nc.scalar.activation(sig[:, 0:256], ps[:, 0:256],
                                     mybir.ActivationFunctionType.Sigmoid)
                nc.scalar.activation(ct[:], ps[:, 384:512],
                                     mybir.ActivationFunctionType.Tanh)
                nc.scalar.activation(sig[:, 256:384], ps[:, 256:384],
                                     mybir.ActivationFunctionType.Sigmoid)
                t1 = tpool.tile([B, 128], f32, tag="t1")
                t2 = tpool.tile([B, 128], f32, tag="t2")
                nc.vector.tensor_mul(t1[:], sig[:, 128:256], C[:])
                nc.vector.tensor_mul(t2[:], sig[:, 0:128], ct[:])
                nc.vector.tensor_add(C[:], t1[:], t2[:])
                tC = tpool.tile([B, 128], f32, tag="tC")
                nc.scalar.activation(tC[:], C[:], mybir.ActivationFunctionType.Tanh)
                Hn = tpool.tile([B, 128], f32, tag="Hn")
                nc.vector.tensor_mul(Hn[:], sig[:, 256:384], tC[:])
                pT = psT.tile([128, B], f32, tag="psT")
                nc.tensor.transpose(pT[:], Hn[:], ident[:])
                nc.vector.tensor_copy(HT[:], pT[:])
                if t % 8 == 0:
                    ppred = psP.tile([1, 8 * B], f32, tag="psP")
                nc.tensor.matmul(ppred[:, (t % 8) * B:(t % 8 + 1) * B], wd[:],
                                 HT[:], start=True, stop=True)
                if t % 8 == 7:
                    prow = tpool.tile([1, 8 * B], f32, tag="prow")
                    nc.vector.tensor_copy(prow[:], ppred[:])
                    nc.sync.dma_start(predp_d[t // 8:t // 8 + 1, :], prow[:])
                if t < steps - 1:
                    ag_in = dinpool.tile([128, B], bf16, tag="agin")
                    nc.sync.dma_start(ag_in[:], HT[:])
                    nc.gpsimd.collective_compute(
                        "AllGather", mybir.AluOpType.bypass,
                        replica_groups=[list(range(ncores))],
                        ins=[ag_in.opt()], outs=[ag_outs[t].opt()],
                    )
                    G = gpool.tile([128, 8 * B], bf16, tag="G", name=f"G_{t}")
                    src_ap = ag_outs[t][:].rearrange("(p r) b -> p (r b)", p=128)
                    nc.scalar.dma_start(G[:], src_ap)
            nc.sync.dma_start(hf_d[:], Hn[:])
            nc.sync.dma_start(cf_d[:], C[:])
    nc.compile()
    return nc


def _shard_inputs(inputs, ncores=R):
    x = np.asarray(inputs["inputs"], np.float32)
    H0 = np.asarray(inputs["H0"], np.float32)
    C0 = np.asarray(inputs["C0"], np.float32)
    Wx = np.concatenate([np.asarray(inputs[k], np.float32) for k in
                         ("Wxi", "Wxf", "Wxo", "Wxc")], axis=1)
    Wh = np.concatenate([np.asarray(inputs[k], np.float32) for k in
                         ("Whi", "Whf", "Who", "Whc")], axis=1)
    bfull = np.concatenate([np.asarray(inputs[k], np.float32) for k in
                            ("bi", "bf", "bo", "bc")])
    Wd = np.asarray(inputs["Wd"], np.float32)

    def cast(a):
        return a.astype(ml_dtypes.bfloat16)

    xt = np.transpose(x, (1, 2, 0)).reshape(S, 2, 128, B).transpose(0, 2, 1, 3)
    xt = cast(np.ascontiguousarray(xt.reshape(S, 128, 2 * B)))
    # h0t[p, r*B + b] = H0[b, 8p + r] (matches the contiguous-import layout)
    h0t = cast(np.ascontiguousarray(
        H0.T.reshape(128, 8, B).transpose(0, 1, 2).reshape(128, 8 * B)))
    ones = cast(np.ones((1, B), np.float32))
    ident = np.eye(B, dtype=np.float32)

    in_maps = []
    for r in range(ncores):
        cols = np.concatenate([np.arange(gi * H + r * HS, gi * H + (r + 1) * HS)
                               for gi in range(4)])
        Wh_loc = Wh[:, cols]
        Wx_loc = Wx[:, cols]
        wh_chunks = np.zeros((10, 128, NL), np.float32)
        for r8 in range(8):
            # gather tile slice r8 holds hidden units 8p + r8 (p = partition)
            wh_chunks[r8] = Wh_loc[r8::8]
        wh_chunks[8] = Wx_loc[0:128]
        wh_chunks[9] = Wx_loc[128:256]
        in_maps.append({
            "wh": cast(wh_chunks),
            "bvec": cast(bfull[cols][None, :]),
            "ones": ones,
            "ident": ident,
            "wd": cast(Wd[r * HS:(r + 1) * HS, :]),
            "h0t": h0t,
            "c0": np.ascontiguousarray(C0[:, r * HS:(r + 1) * HS]),
            "xt": xt,
        })
    return in_maps


def _assemble(results, inputs):
    bd = np.asarray(inputs["bd"], np.float32)
    predp = np.zeros((S // 8, 8 * B), np.float32)
    for r in range(len(results)):
        predp += results[r]["predp"]
    pred = predp.reshape(S // 8, 8, B).transpose(2, 0, 1).reshape(B, S)
    pred = (pred + bd[0])[:, :, None].astype(np.float32)
    Hf = np.concatenate([results[r]["hf"] for r in range(len(results))], axis=1)
    Cf = np.concatenate([results[r]["cf"] for r in range(len(results))], axis=1)
    return pred, (Hf.astype(np.float32), Cf.astype(np.float32))


_NC_CACHE = {}


def kernel(**inputs):
    if "nc" not in _NC_CACHE:
        _NC_CACHE["nc"] = _build()
    nc = _NC_CACHE["nc"]
    in_maps = _shard_inputs(inputs)
    res = run_bass_kernel_spmd(nc, in_maps, core_ids=list(range(R)))
    return _assemble(res.results, inputs)
